# revision 9
# baseline (speedup 1.0000x reference)
"""Trainium2 Bass kernel for a sliding-window self-attention block.

The reference network applies softmax over a singleton axis, so the attention
weights are identically 1.0 and the whole module reduces to:

    h   = relu((x + pos_enc) @ W1 + b1)            # [B, S, 64]
    ws  = sliding_window_sum(h, +-8, zero-padded)  # [B, S, 64]
    out = ws @ (Wv @ W2) + b2                      # [B, S, 2]
    weights = ones([B, S, 1, 17])

(The window sum commutes with the trailing matmul, so we apply Wc = Wv@W2
per-token first and window-sum the tiny [*, 2] result.)

Sharding: data-parallel over batch, 2 batches per core on 8 NeuronCores.
"""

import numpy as np

B, S, I, H, O = 16, 4096, 64, 64, 2
A = 8                 # atten_size; window = 2*A+1 = 17
NCORES = 8
BPC = B // NCORES     # batches per core = 2
CHUNK = 512           # seq positions per inner chunk
NCHUNK = S // CHUNK   # 8
PAD = 4240            # 8 (left zero pad) + 4096 + 136 (right pad/tail slack)

_PROGRAM = None


def _build_program():
    import concourse.bacc as bacc
    import concourse.mybir as mybir
    from concourse.tile import TileContext

    f32 = mybir.dt.float32
    f32r = mybir.dt.float32r

    nc = bacc.Bacc()

    x_d = nc.declare_dram_parameter("x", [BPC, S, I], f32, isOutput=False)
    p_d = nc.declare_dram_parameter("p", [BPC, S, I], f32, isOutput=False)
    w1_d = nc.declare_dram_parameter("w1blk", [128, 128], f32r, isOutput=False)
    wc_d = nc.declare_dram_parameter("wcblk", [128, 2 * BPC], f32r, isOutput=False)
    b1_d = nc.declare_dram_parameter("b1v", [128, 1], f32, isOutput=False)
    b2_d = nc.declare_dram_parameter("b2v", [128, 1], f32, isOutput=False)
    id_d = nc.declare_dram_parameter("ident", [128, 128], f32r, isOutput=False)
    o_d = nc.declare_dram_parameter("o", [2 * BPC, S], f32, isOutput=True)

    with TileContext(nc) as tc:
        with (
            tc.tile_pool(name="const", bufs=1) as const,
            tc.tile_pool(name="inp", bufs=3) as inp,
            tc.tile_pool(name="h0p", bufs=3) as h0p,
            tc.tile_pool(name="hsb", bufs=3) as hsb,
            tc.tile_pool(name="pp", bufs=1) as pp,
            tc.tile_pool(name="wsum", bufs=2) as wsum,
            tc.tile_pool(name="ps_t", bufs=2, space="PSUM") as ps_t,
            tc.tile_pool(name="ps_h", bufs=2, space="PSUM") as ps_h,
            tc.tile_pool(name="ps_p", bufs=2, space="PSUM") as ps_p,
        ):
            w1_t = const.tile([128, 128], f32r)
            nc.sync.dma_start(out=w1_t[:], in_=w1_d[:])
            wc_t = const.tile([128, 2 * BPC], f32r)
            nc.sync.dma_start(out=wc_t[:], in_=wc_d[:])
            b1_t = const.tile([128, 1], f32)
            nc.sync.dma_start(out=b1_t[:], in_=b1_d[:])
            b2_t = const.tile([128, 1], f32)
            nc.sync.dma_start(out=b2_t[:], in_=b2_d[:])
            id_t = const.tile([128, 128], f32r)
            nc.sync.dma_start(out=id_t[:], in_=id_d[:])

            # p_pad[(2b+f), 8 + s] = p^T values; zero pads at both ends so the
            # halo gather below never needs edge cases.
            p_pad = pp.tile([2 * BPC, PAD], f32)
            nc.vector.memset(p_pad[:, 0:8], 0.0)
            nc.vector.memset(p_pad[:, 8 + S : PAD], 0.0)

            J = CHUNK // 128  # 4 transpose tiles per chunk
            for k in range(NCHUNK):
                s0 = k * CHUNK
                # h0c[q, j, b, f] = x + pos for seq position s0 + 128*j + q,
                # batch b, feature f: each [q, j-slice] is a [128, 128] tile
                # whose free dim is (b, f) — transposing it puts (b, f) on
                # partitions, giving the block-diagonal layout directly.
                h0c = inp.tile([128, J, BPC, I], f32r, tag="h0")
                for b in range(BPC):
                    xt = inp.tile([128, J, I], f32, tag="xt")
                    nc.sync.dma_start(
                        out=xt[:],
                        in_=x_d[b, s0 : s0 + CHUNK, :].rearrange(
                            "(j q) f -> q j f", q=128
                        ),
                    )
                    pt = inp.tile([128, J, I], f32, tag="pt")
                    nc.sync.dma_start(
                        out=pt[:],
                        in_=p_d[b, s0 : s0 + CHUNK, :].rearrange(
                            "(j q) f -> q j f", q=128
                        ),
                    )
                    nc.vector.tensor_add(out=h0c[:, :, b, :], in0=xt[:], in1=pt[:])

                # Transpose into one [128, CHUNK] PSUM tile: partition = (b, f).
                h0T_ps = ps_t.tile([128, CHUNK], f32r)
                for j in range(J):
                    nc.tensor.transpose(
                        out=h0T_ps[:, 128 * j : 128 * j + 128],
                        in_=h0c[:, j, :, :],
                        identity=id_t[:],
                    )
                h0T = hsb.tile([128, CHUNK], f32r, tag="h0T")
                nc.vector.tensor_copy(out=h0T[:], in_=h0T_ps[:])

                # h^T = blockdiag(W1, W1)^T-contraction over 128 partitions.
                hT_ps = ps_h.tile([128, CHUNK], f32)
                nc.tensor.matmul(
                    out=hT_ps[:], lhsT=w1_t[:], rhs=h0T[:], start=True, stop=True
                )
                hT = hsb.tile([128, CHUNK], f32r, tag="hT")
                nc.scalar.activation(
                    out=hT[:],
                    in_=hT_ps[:],
                    func=mybir.ActivationFunctionType.Relu,
                    bias=b1_t[:],
                )

                # p^T = blockdiag(Wc, Wc) contraction -> [4, CHUNK]
                pT_ps = ps_p.tile([2 * BPC, CHUNK], f32)
                nc.tensor.matmul(
                    out=pT_ps[:], lhsT=wc_t[:], rhs=hT[:], start=True, stop=True
                )
                nc.scalar.copy(out=p_pad[:, 8 + s0 : 8 + s0 + CHUNK], in_=pT_ps[:])

            # Re-partition into halo layout Q[(bf,c), u] = p^T[bf, 128c + u - 8]
            # (zero-padded), bf in [0,4), c in [0,32), u in [0,144).
            q_t = wsum.tile([128, 144], f32, tag="q")
            for m in range(2 * BPC):
                row = q_t[32 * m : 32 * m + 32, :]
                nc.sync.dma_start(
                    out=row[:, 0:128],
                    in_=p_pad[m : m + 1, 0 : 128 * 32].rearrange(
                        "p (c u) -> p c u", u=128
                    ),
                )
                nc.sync.dma_start(
                    out=row[:, 128:136],
                    in_=p_pad[m : m + 1, 128 : 128 + 128 * 32].rearrange(
                        "p (c u) -> p c u", u=128
                    )[:, :, 0:8],
                )
                nc.sync.dma_start(
                    out=row[:, 136:144],
                    in_=p_pad[m : m + 1, 136 : 136 + 128 * 32].rearrange(
                        "p (c u) -> p c u", u=128
                    )[:, :, 0:8],
                )

            # 17-wide window sum via doubling: ws[u] = sum_{d=0..16} Q[u+d].
            t2 = wsum.tile([128, 143], f32, tag="t2")
            nc.vector.tensor_add(out=t2[:], in0=q_t[:, 0:143], in1=q_t[:, 1:144])
            t4 = wsum.tile([128, 141], f32, tag="t4")
            nc.vector.tensor_add(out=t4[:], in0=t2[:, 0:141], in1=t2[:, 2:143])
            t8 = wsum.tile([128, 137], f32, tag="t8")
            nc.vector.tensor_add(out=t8[:], in0=t4[:, 0:137], in1=t4[:, 4:141])
            t16 = wsum.tile([128, 129], f32, tag="t16")
            nc.vector.tensor_add(out=t16[:], in0=t8[:, 0:129], in1=t8[:, 8:137])
            ws_t = wsum.tile([128, 128], f32, tag="ws")
            nc.vector.tensor_add(out=ws_t[:], in0=t16[:, 0:128], in1=q_t[:, 16:144])

            # + b2 (per-partition bias), then store: o[bf, 128c + u] = ows[(bf,c), u]
            ows = wsum.tile([128, 128], f32, tag="ows")
            nc.scalar.activation(
                out=ows[:],
                in_=ws_t[:],
                func=mybir.ActivationFunctionType.Identity,
                bias=b2_t[:],
            )
            nc.sync.dma_start(
                out=o_d[:, :].rearrange("p (c u) -> (p c) u", u=128), in_=ows[:]
            )

    nc.finalize()
    return nc


def _get_program():
    global _PROGRAM
    if _PROGRAM is None:
        _PROGRAM = _build_program()
    return _PROGRAM


def _host_inputs(W1, b1, Wv, W2, b2):
    """Build the small replicated parameter tensors."""
    W1 = np.asarray(W1, np.float32).reshape(I, H)
    Wc = (np.asarray(Wv, np.float32).reshape(H, H) @ np.asarray(W2, np.float32).reshape(H, O)).astype(np.float32)
    w1blk = np.zeros((128, 128), np.float32)
    w1blk[:64, :64] = W1
    w1blk[64:, 64:] = W1
    wcblk = np.zeros((128, 2 * BPC), np.float32)
    wcblk[:64, :O] = Wc
    wcblk[64:, O : 2 * O] = Wc
    b1v = np.tile(np.asarray(b1, np.float32).reshape(H), BPC).reshape(128, 1).copy()
    # partition (b, f, c) = 64*b + 32*f + c  ->  f = (p % 64) // 32
    pidx = np.arange(128)
    b2v = np.asarray(b2, np.float32).reshape(O)[(pidx % 64) // 32].reshape(128, 1).copy()
    ident = np.eye(128, dtype=np.float32)
    return w1blk, wcblk, b1v, b2v, ident


def _in_maps(x, pos_enc, W1, b1, Wv, W2, b2):
    x = np.asarray(x, np.float32)
    pos_enc = np.asarray(pos_enc, np.float32)
    w1blk, wcblk, b1v, b2v, ident = _host_inputs(W1, b1, Wv, W2, b2)
    in_maps = []
    for r in range(NCORES):
        sl = slice(r * BPC, (r + 1) * BPC)
        in_maps.append(
            {
                "x": np.ascontiguousarray(x[sl]),
                "p": np.ascontiguousarray(pos_enc[sl]),
                "w1blk": w1blk,
                "wcblk": wcblk,
                "b1v": b1v,
                "b2v": b2v,
                "ident": ident,
            }
        )
    return in_maps


def _assemble(results):
    out = np.empty((B, S, O), np.float32)
    for r in range(NCORES):
        o = np.asarray(results[r]["o"]).reshape(BPC, O, S)
        out[r * BPC : (r + 1) * BPC] = o.transpose(0, 2, 1)
    weights = np.ones((B, S, 1, 2 * A + 1), np.float32)
    return out, weights


def kernel(x, pos_enc, W1, b1, Wq, Wk, Wv, W2, b2):
    from concourse.bass_utils import run_bass_kernel_spmd

    in_maps = _in_maps(x, pos_enc, W1, b1, Wv, W2, b2)
    nc = _get_program()
    res = run_bass_kernel_spmd(nc, in_maps, list(range(NCORES))).results
    return _assemble(res)


# revision 13
# speedup vs baseline: 1.2754x; 1.2754x over previous
"""Trainium2 Bass kernel for a sliding-window self-attention block.

The reference network applies softmax over a singleton axis, so the attention
weights are identically 1.0 and the whole module reduces to:

    h   = relu((x + pos_enc) @ W1 + b1)            # [B, S, 64]
    p   = h @ (Wv @ W2)                            # [B, S, 2]
    out = sliding_window_sum(p, +-8, zero-pad) + b2
    weights = ones([B, S, 1, 17])

Sharding: data-parallel over batch, 2 batches per core on 8 NeuronCores.

Per-core dataflow (batches b0/b1 processed jointly on 128 partitions):
  - 4 big DMAs load x and pos_enc (1 MB each, both batches folded).
  - DVE adds -> h0 in [seq_on_partitions, (j, b, feat)] layout (float32r).
  - PE transposes [128 seq, (b,f)=128] tiles -> PSUM [(b,f), seq] (bf16
    identity as the moving operand: 1 cycle/row).
  - One block-diagonal f32r matmul per 512-chunk applies W1 to both batches,
    ACT relu(+b1) -> one more block-diag matmul applies Wc = Wv@W2 -> p^T.
  - p^T chunks gather into a zero-padded [4, 4240] buffer; 3 SBUF DMAs
    re-partition it into a [128(b,f,c), 144] halo layout; 5 log-tree DVE
    adds compute the 17-wide window sum; ACT adds b2; one DMA stores
    [4, 4096] = out^T per batch (host transposes the tiny result).
"""

import numpy as np

B, S, I, H, O = 16, 4096, 64, 64, 2
A = 8                 # atten_size; window = 2*A+1 = 17
NCORES = 8
BPC = B // NCORES     # batches per core = 2
CHUNK = 512           # seq positions per PSUM-stage chunk
NCHUNK = S // CHUNK   # 8
GCHUNK = 2048         # seq positions per DMA load chunk
NG = S // GCHUNK      # 2
JG = GCHUNK // 128    # 16 transpose tiles per load chunk
PAD = 4240            # 8 (left zero pad) + 4096 + 136 (right pad/tail slack)

_PROGRAM = None


def _build_program():
    import concourse.bacc as bacc
    import concourse.mybir as mybir
    from concourse.tile import TileContext

    f32 = mybir.dt.float32
    f32r = mybir.dt.float32r
    bf16 = mybir.dt.bfloat16

    nc = bacc.Bacc()

    x_d = nc.declare_dram_parameter("x", [BPC, S, I], f32, isOutput=False)
    p_d = nc.declare_dram_parameter("p", [BPC, S, I], f32, isOutput=False)
    c_d = nc.declare_dram_parameter("consts", [128, 262], f32r, isOutput=False)
    o_d = nc.declare_dram_parameter("o", [2 * BPC, S], f32, isOutput=True)

    with TileContext(nc) as tc:
        with (
            tc.tile_pool(name="const", bufs=1) as const,
            tc.tile_pool(name="inp", bufs=2) as inp,
            tc.tile_pool(name="hsb", bufs=3) as hsb,
            tc.tile_pool(name="pp", bufs=1) as pp,
            tc.tile_pool(name="wsum", bufs=2) as wsum,
            tc.tile_pool(name="ps_t", bufs=2, space="PSUM") as ps_t,
            tc.tile_pool(name="ps_h", bufs=2, space="PSUM") as ps_h,
            tc.tile_pool(name="ps_p", bufs=2, space="PSUM") as ps_p,
        ):
            c_t = const.tile([128, 262], f32r)
            nc.sync.dma_start(out=c_t[:], in_=c_d[:])
            w1_t = c_t[:, 0:128]
            wc_t = c_t[:, 128 : 128 + 2 * BPC]
            id_t = c_t[:, 132:260]                    # [128, 128] f32r identity
            b1_t = c_t[:, 260:261].bitcast(f32)
            b2_t = c_t[:, 261:262].bitcast(f32)

            # p_pad[(2b+f), 8 + s] = p^T values; zero pads at both ends so the
            # halo gather below never needs edge cases.
            p_pad = pp.tile([2 * BPC, PAD], f32)
            nc.vector.memset(p_pad[:, 0:8], 0.0)
            nc.vector.memset(p_pad[:, 8 + S : PAD], 0.0)

            for g in range(NG):
                s0 = g * GCHUNK
                xt = inp.tile([128, BPC, JG, I], f32, tag="xt")
                pt = inp.tile([128, BPC, JG, I], f32, tag="pt")
                for b in range(BPC):
                    nc.sync.dma_start(
                        out=xt[:, b],
                        in_=x_d[b, s0 : s0 + GCHUNK, :].rearrange(
                            "(j q) f -> q j f", q=128
                        ),
                    )
                    nc.sync.dma_start(
                        out=pt[:, b],
                        in_=p_d[b, s0 : s0 + GCHUNK, :].rearrange(
                            "(j q) f -> q j f", q=128
                        ),
                    )
                # h0c[q, j, b, f] = x + pos; each j-slice is a [128, 128] tile
                # whose free dim is (b, f) — transposing puts (b, f) on
                # partitions, which is exactly the block-diagonal layout.
                h0c = inp.tile([128, JG, BPC, I], f32r, tag="h0")
                for b in range(BPC):
                    nc.vector.tensor_add(
                        out=h0c[:, :, b, :], in0=xt[:, b], in1=pt[:, b]
                    )

                for kk in range(GCHUNK // CHUNK):
                    s1 = s0 + kk * CHUNK
                    j0 = kk * (CHUNK // 128)
                    h0T_ps = ps_t.tile([128, CHUNK], f32r)
                    for j in range(CHUNK // 128):
                        nc.tensor.transpose(
                            out=h0T_ps[:, 128 * j : 128 * j + 128],
                            in_=h0c[:, j0 + j, :, :],
                            identity=id_t[:],
                        )
                    h0T = hsb.tile([128, CHUNK], f32r, tag="h0T")
                    nc.vector.tensor_copy(out=h0T[:], in_=h0T_ps[:])

                    hT_ps = ps_h.tile([128, CHUNK], f32)
                    nc.tensor.matmul(
                        out=hT_ps[:], lhsT=w1_t, rhs=h0T[:], start=True, stop=True
                    )
                    hT = hsb.tile([128, CHUNK], f32r, tag="hT")
                    nc.scalar.activation(
                        out=hT[:],
                        in_=hT_ps[:],
                        func=mybir.ActivationFunctionType.Relu,
                        bias=b1_t,
                    )

                    pT_ps = ps_p.tile([2 * BPC, CHUNK], f32)
                    nc.tensor.matmul(
                        out=pT_ps[:], lhsT=wc_t, rhs=hT[:], start=True, stop=True
                    )
                    nc.scalar.copy(
                        out=p_pad[:, 8 + s1 : 8 + s1 + CHUNK], in_=pT_ps[:]
                    )

            # Re-partition into halo layout Q[(bf,c), u] = p^T[bf, 128c + u - 8]
            # (zero-padded), bf in [0,4), c in [0,32), u in [0,144).
            q_t = wsum.tile([128, 144], f32, tag="q")
            nc.sync.dma_start(
                out=q_t[:, 0:128],
                in_=p_pad[:, 0 : 128 * 32].rearrange("p (c u) -> p c u", u=128),
            )
            nc.sync.dma_start(
                out=q_t[:, 128:136],
                in_=p_pad[:, 128 : 128 + 128 * 32].rearrange(
                    "p (c u) -> p c u", u=128
                )[:, :, 0:8],
            )
            nc.sync.dma_start(
                out=q_t[:, 136:144],
                in_=p_pad[:, 136 : 136 + 128 * 32].rearrange(
                    "p (c u) -> p c u", u=128
                )[:, :, 0:8],
            )

            # 17-wide window sum via doubling: ws[u] = sum_{d=0..16} Q[u+d].
            t2 = wsum.tile([128, 143], f32, tag="t2")
            nc.vector.tensor_add(out=t2[:], in0=q_t[:, 0:143], in1=q_t[:, 1:144])
            t4 = wsum.tile([128, 141], f32, tag="t4")
            nc.vector.tensor_add(out=t4[:], in0=t2[:, 0:141], in1=t2[:, 2:143])
            t8 = wsum.tile([128, 137], f32, tag="t8")
            nc.vector.tensor_add(out=t8[:], in0=t4[:, 0:137], in1=t4[:, 4:141])
            t16 = wsum.tile([128, 129], f32, tag="t16")
            nc.vector.tensor_add(out=t16[:], in0=t8[:, 0:129], in1=t8[:, 8:137])
            ws_t = wsum.tile([128, 128], f32, tag="ws")
            nc.vector.tensor_add(out=ws_t[:], in0=t16[:, 0:128], in1=q_t[:, 16:144])

            # + b2 (per-partition bias), then store: o[bf, 128c + u]
            ows = wsum.tile([128, 128], f32, tag="ows")
            nc.scalar.activation(
                out=ows[:],
                in_=ws_t[:],
                func=mybir.ActivationFunctionType.Identity,
                bias=b2_t,
            )
            nc.sync.dma_start(
                out=o_d[:, :].rearrange("p (c u) -> (p c) u", u=128), in_=ows[:]
            )

    nc.finalize()
    return nc


def _get_program():
    global _PROGRAM
    if _PROGRAM is None:
        _PROGRAM = _build_program()
    return _PROGRAM


def _host_inputs(W1, b1, Wv, W2, b2):
    """Pack the small replicated parameters into one [128, 198] f32 tensor."""
    W1 = np.asarray(W1, np.float32).reshape(I, H)
    Wc = (
        np.asarray(Wv, np.float32).reshape(H, H)
        @ np.asarray(W2, np.float32).reshape(H, O)
    ).astype(np.float32)
    consts = np.zeros((128, 262), np.float32)
    consts[:64, 0:64] = W1
    consts[64:, 64:128] = W1
    consts[:64, 128 : 128 + O] = Wc
    consts[64:, 128 + O : 128 + 2 * O] = Wc
    consts[:, 132:260] = np.eye(128, dtype=np.float32)
    consts[:, 260] = np.tile(np.asarray(b1, np.float32).reshape(H), BPC)
    pidx = np.arange(128)
    consts[:, 261] = np.asarray(b2, np.float32).reshape(O)[(pidx % 64) // 32]
    return consts


def _in_maps(x, pos_enc, W1, b1, Wv, W2, b2):
    x = np.asarray(x, np.float32)
    pos_enc = np.asarray(pos_enc, np.float32)
    consts = _host_inputs(W1, b1, Wv, W2, b2)
    in_maps = []
    for r in range(NCORES):
        sl = slice(r * BPC, (r + 1) * BPC)
        in_maps.append(
            {
                "x": np.ascontiguousarray(x[sl]),
                "p": np.ascontiguousarray(pos_enc[sl]),
                "consts": consts,
            }
        )
    return in_maps


def _assemble(results):
    out = np.empty((B, S, O), np.float32)
    for r in range(NCORES):
        o = np.asarray(results[r]["o"]).reshape(BPC, O, S)
        out[r * BPC : (r + 1) * BPC] = o.transpose(0, 2, 1)
    weights = np.ones((B, S, 1, 2 * A + 1), np.float32)
    return out, weights


def kernel(x, pos_enc, W1, b1, Wq, Wk, Wv, W2, b2):
    from concourse.bass_utils import run_bass_kernel_spmd

    in_maps = _in_maps(x, pos_enc, W1, b1, Wv, W2, b2)
    nc = _get_program()
    res = run_bass_kernel_spmd(nc, in_maps, list(range(NCORES))).results
    return _assemble(res)


# revision 16
# speedup vs baseline: 1.3829x; 1.0843x over previous
"""Trainium2 Bass kernel for a sliding-window self-attention block.

The reference network applies softmax over a singleton axis, so the attention
weights are identically 1.0 and the whole module reduces to:

    h   = relu((x + pos_enc) @ W1 + b1)            # [B, S, 64]
    p   = h @ (Wv @ W2)                            # [B, S, 2]
    out = sliding_window_sum(p, +-8, zero-pad) + b2
    weights = ones([B, S, 1, 17])

Sharding: data-parallel over batch, 2 batches per core on 8 NeuronCores.

Per-core dataflow (batches b0/b1 processed jointly on 128 partitions):
  - 4 big DMAs load x and pos_enc (1 MB each, both batches folded).
  - DVE adds -> h0 in [seq_on_partitions, (j, b, feat)] layout (float32r).
  - PE transposes [128 seq, (b,f)=128] tiles -> PSUM [(b,f), seq] (bf16
    identity as the moving operand: 1 cycle/row).
  - One block-diagonal f32r matmul per 512-chunk applies W1 to both batches,
    ACT relu(+b1) -> one more block-diag matmul applies Wc = Wv@W2 -> p^T.
  - p^T chunks gather into a zero-padded [4, 4240] buffer; 3 SBUF DMAs
    re-partition it into a [128(b,f,c), 144] halo layout; 5 log-tree DVE
    adds compute the 17-wide window sum; ACT adds b2; one DMA stores
    [4, 4096] = out^T per batch (host transposes the tiny result).
"""

import numpy as np

B, S, I, H, O = 16, 4096, 64, 64, 2
A = 8                 # atten_size; window = 2*A+1 = 17
NCORES = 8
BPC = B // NCORES     # batches per core = 2
CHUNK = 512           # seq positions per PSUM-stage chunk
NCHUNK = S // CHUNK   # 8
GCHUNK = 2048         # seq positions per DMA load chunk
NG = S // GCHUNK      # 2
JG = GCHUNK // 128    # 16 transpose tiles per load chunk
PAD = 4240            # 8 (left zero pad) + 4096 + 136 (right pad/tail slack)

_PROGRAM = None


def _build_program():
    import concourse.bacc as bacc
    import concourse.mybir as mybir
    from concourse.tile import TileContext

    f32 = mybir.dt.float32
    f32r = mybir.dt.float32r
    bf16 = mybir.dt.bfloat16

    nc = bacc.Bacc()

    x_d = nc.declare_dram_parameter("x", [BPC, S, I], f32, isOutput=False)
    p_d = nc.declare_dram_parameter("p", [BPC, S, I], f32, isOutput=False)
    c_d = nc.declare_dram_parameter("consts", [128, 262], f32r, isOutput=False)
    o_d = nc.declare_dram_parameter("o", [2 * BPC, S], f32, isOutput=True)

    with TileContext(nc) as tc:
        with (
            tc.tile_pool(name="const", bufs=1) as const,
            tc.tile_pool(name="inp", bufs=2) as inp,
            tc.tile_pool(name="hsb", bufs=3) as hsb,
            tc.tile_pool(name="pp", bufs=1) as pp,
            tc.tile_pool(name="wsum", bufs=2) as wsum,
            tc.tile_pool(name="ps_t", bufs=2, space="PSUM") as ps_t,
            tc.tile_pool(name="ps_h", bufs=2, space="PSUM") as ps_h,
            tc.tile_pool(name="ps_p", bufs=2, space="PSUM") as ps_p,
        ):
            c_t = const.tile([128, 262], f32r)
            nc.sync.dma_start(out=c_t[:], in_=c_d[:])
            w1_t = c_t[:, 0:128]
            wc_t = c_t[:, 128 : 128 + 2 * BPC]
            id_t = c_t[:, 132:260]                    # [128, 128] f32r identity
            b1_t = c_t[:, 260:261].bitcast(f32)
            b2_t = c_t[:, 261:262].bitcast(f32)

            # p_pad[(2b+f), 8 + s] = p^T values; zero pads at both ends so the
            # halo gather below never needs edge cases.
            p_pad = pp.tile([2 * BPC, PAD], f32)
            nc.vector.memset(p_pad[:, 0:8], 0.0)
            nc.vector.memset(p_pad[:, 8 + S : PAD], 0.0)

            for g in range(NG):
                s0 = g * GCHUNK
                # Raw layout: partition q holds 16 consecutive seq rows
                # (4 KB contiguous per partition -> cheap DMA descriptors).
                # Element (q, b, 64u + f) = x[b, s0 + 16q + u, f].
                xt = inp.tile([128, BPC, GCHUNK // 2], f32, tag="xt")
                pt = inp.tile([128, BPC, GCHUNK // 2], f32, tag="pt")
                for b in range(BPC):
                    nc.sync.dma_start(
                        out=xt[:, b],
                        in_=x_d[b, s0 : s0 + GCHUNK, :].rearrange(
                            "(q v) f -> q (v f)", q=128
                        ),
                    )
                    nc.scalar.dma_start(
                        out=pt[:, b],
                        in_=p_d[b, s0 : s0 + GCHUNK, :].rearrange(
                            "(q v) f -> q (v f)", q=128
                        ),
                    )
                # h0c free layout (u, b, f): the (b, f) pair of each u-slice is
                # contiguous, so the transpose stationary is a single free dim.
                h0c = inp.tile([128, JG, BPC, I], f32r, tag="h0")
                nc.vector.tensor_add(
                    out=h0c[:, :, 0, :],
                    in0=xt[:, 0].rearrange("q (v f) -> q v f", f=I),
                    in1=pt[:, 0].rearrange("q (v f) -> q v f", f=I),
                )
                nc.gpsimd.tensor_add(
                    out=h0c[:, :, 1, :],
                    in0=xt[:, 1].rearrange("q (v f) -> q v f", f=I),
                    in1=pt[:, 1].rearrange("q (v f) -> q v f", f=I),
                )

                for kk in range(GCHUNK // CHUNK):
                    # u-slices 4kk..4kk+3; transpose input [128, (b, f)] whose
                    # column q maps to seq s0 + 16q + u.
                    h0T_ps = ps_t.tile([128, CHUNK], f32r)
                    for ul in range(CHUNK // 128):
                        u = 4 * kk + ul
                        nc.tensor.transpose(
                            out=h0T_ps[:, 128 * ul : 128 * ul + 128],
                            in_=h0c[:, u].rearrange("p b f -> p (b f)"),
                            identity=id_t[:],
                        )
                    h0T = hsb.tile([128, CHUNK], f32r, tag="h0T")
                    nc.vector.tensor_copy(out=h0T[:], in_=h0T_ps[:])

                    hT_ps = ps_h.tile([128, CHUNK], f32)
                    nc.tensor.matmul(
                        out=hT_ps[:], lhsT=w1_t, rhs=h0T[:], start=True, stop=True
                    )
                    hT = hsb.tile([128, CHUNK], f32r, tag="hT")
                    nc.scalar.activation(
                        out=hT[:],
                        in_=hT_ps[:],
                        func=mybir.ActivationFunctionType.Relu,
                        bias=b1_t,
                    )

                    pT_ps = ps_p.tile([2 * BPC, CHUNK], f32)
                    nc.tensor.matmul(
                        out=pT_ps[:], lhsT=wc_t, rhs=hT[:], start=True, stop=True
                    )
                    # Un-permute while scattering: pT col (ul, q) is seq
                    # s0 + 16q + 4kk + ul -> p_pad col base + 16q + ul.
                    base = 8 + s0 + 4 * kk
                    nc.scalar.copy(
                        out=p_pad[:, base : base + 2048].rearrange(
                            "p (pp u) -> p u pp", u=16
                        )[:, 0:4, :],
                        in_=pT_ps.rearrange("p (u q) -> p u q", q=128),
                    )

            # Re-partition into halo layout Q[(bf,c), u] = p^T[bf, 128c + u - 8]
            # (zero-padded), bf in [0,4), c in [0,32), u in [0,144).
            q_t = wsum.tile([128, 144], f32, tag="q")
            nc.sync.dma_start(
                out=q_t[:, 0:128],
                in_=p_pad[:, 0 : 128 * 32].rearrange("p (c u) -> p c u", u=128),
            )
            nc.sync.dma_start(
                out=q_t[:, 128:136],
                in_=p_pad[:, 128 : 128 + 128 * 32].rearrange(
                    "p (c u) -> p c u", u=128
                )[:, :, 0:8],
            )
            nc.sync.dma_start(
                out=q_t[:, 136:144],
                in_=p_pad[:, 136 : 136 + 128 * 32].rearrange(
                    "p (c u) -> p c u", u=128
                )[:, :, 0:8],
            )

            # 17-wide window sum via doubling: ws[u] = sum_{d=0..16} Q[u+d].
            t2 = wsum.tile([128, 143], f32, tag="t2")
            nc.vector.tensor_add(out=t2[:], in0=q_t[:, 0:143], in1=q_t[:, 1:144])
            t4 = wsum.tile([128, 141], f32, tag="t4")
            nc.vector.tensor_add(out=t4[:], in0=t2[:, 0:141], in1=t2[:, 2:143])
            t8 = wsum.tile([128, 137], f32, tag="t8")
            nc.vector.tensor_add(out=t8[:], in0=t4[:, 0:137], in1=t4[:, 4:141])
            t16 = wsum.tile([128, 129], f32, tag="t16")
            nc.vector.tensor_add(out=t16[:], in0=t8[:, 0:129], in1=t8[:, 8:137])
            ws_t = wsum.tile([128, 128], f32, tag="ws")
            nc.vector.tensor_add(out=ws_t[:], in0=t16[:, 0:128], in1=q_t[:, 16:144])

            # + b2 (per-partition bias), then store: o[bf, 128c + u]
            ows = wsum.tile([128, 128], f32, tag="ows")
            nc.scalar.activation(
                out=ows[:],
                in_=ws_t[:],
                func=mybir.ActivationFunctionType.Identity,
                bias=b2_t,
            )
            nc.sync.dma_start(
                out=o_d[:, :].rearrange("p (c u) -> (p c) u", u=128), in_=ows[:]
            )

    nc.finalize()
    return nc


def _get_program():
    global _PROGRAM
    if _PROGRAM is None:
        _PROGRAM = _build_program()
    return _PROGRAM


def _host_inputs(W1, b1, Wv, W2, b2):
    """Pack the small replicated parameters into one [128, 198] f32 tensor."""
    W1 = np.asarray(W1, np.float32).reshape(I, H)
    Wc = (
        np.asarray(Wv, np.float32).reshape(H, H)
        @ np.asarray(W2, np.float32).reshape(H, O)
    ).astype(np.float32)
    consts = np.zeros((128, 262), np.float32)
    consts[:64, 0:64] = W1
    consts[64:, 64:128] = W1
    consts[:64, 128 : 128 + O] = Wc
    consts[64:, 128 + O : 128 + 2 * O] = Wc
    consts[:, 132:260] = np.eye(128, dtype=np.float32)
    consts[:, 260] = np.tile(np.asarray(b1, np.float32).reshape(H), BPC)
    pidx = np.arange(128)
    consts[:, 261] = np.asarray(b2, np.float32).reshape(O)[(pidx % 64) // 32]
    return consts


def _in_maps(x, pos_enc, W1, b1, Wv, W2, b2):
    x = np.asarray(x, np.float32)
    pos_enc = np.asarray(pos_enc, np.float32)
    consts = _host_inputs(W1, b1, Wv, W2, b2)
    in_maps = []
    for r in range(NCORES):
        sl = slice(r * BPC, (r + 1) * BPC)
        in_maps.append(
            {
                "x": np.ascontiguousarray(x[sl]),
                "p": np.ascontiguousarray(pos_enc[sl]),
                "consts": consts,
            }
        )
    return in_maps


def _assemble(results):
    out = np.empty((B, S, O), np.float32)
    for r in range(NCORES):
        o = np.asarray(results[r]["o"]).reshape(BPC, O, S)
        out[r * BPC : (r + 1) * BPC] = o.transpose(0, 2, 1)
    weights = np.ones((B, S, 1, 2 * A + 1), np.float32)
    return out, weights


def kernel(x, pos_enc, W1, b1, Wq, Wk, Wv, W2, b2):
    from concourse.bass_utils import run_bass_kernel_spmd

    in_maps = _in_maps(x, pos_enc, W1, b1, Wv, W2, b2)
    nc = _get_program()
    res = run_bass_kernel_spmd(nc, in_maps, list(range(NCORES))).results
    return _assemble(res)


# revision 18
# speedup vs baseline: 1.4836x; 1.0728x over previous
"""Trainium2 Bass kernel for a sliding-window self-attention block.

The reference network applies softmax over a singleton axis, so the attention
weights are identically 1.0 and the whole module reduces to:

    h   = relu((x + pos_enc) @ W1 + b1)            # [B, S, 64]
    p   = h @ (Wv @ W2)                            # [B, S, 2]
    out = sliding_window_sum(p, +-8, zero-pad) + b2
    weights = ones([B, S, 1, 17])

Sharding: data-parallel over batch, 2 batches per core on 8 NeuronCores.

Per-core dataflow (batches b0/b1 processed jointly on 128 partitions):
  - 4 big DMAs load x and pos_enc (1 MB each, both batches folded).
  - DVE adds -> h0 in [seq_on_partitions, (j, b, feat)] layout (float32r).
  - PE transposes [128 seq, (b,f)=128] tiles -> PSUM [(b,f), seq] (bf16
    identity as the moving operand: 1 cycle/row).
  - One block-diagonal f32r matmul per 512-chunk applies W1 to both batches,
    ACT relu(+b1) -> one more block-diag matmul applies Wc = Wv@W2 -> p^T.
  - p^T chunks gather into a zero-padded [4, 4240] buffer; 3 SBUF DMAs
    re-partition it into a [128(b,f,c), 144] halo layout; 5 log-tree DVE
    adds compute the 17-wide window sum; ACT adds b2; one DMA stores
    [4, 4096] = out^T per batch (host transposes the tiny result).
"""

import numpy as np

B, S, I, H, O = 16, 4096, 64, 64, 2
A = 8                 # atten_size; window = 2*A+1 = 17
NCORES = 8
BPC = B // NCORES     # batches per core = 2
CHUNK = 512           # seq positions per PSUM-stage chunk
NCHUNK = S // CHUNK   # 8
GCHUNK = 2048         # seq positions per DMA load chunk
NG = S // GCHUNK      # 2
JG = GCHUNK // 128    # 16 transpose tiles per load chunk
PAD = 4240            # 8 (left zero pad) + 4096 + 136 (right pad/tail slack)

_PROGRAM = None


def _build_program():
    import concourse.bacc as bacc
    import concourse.mybir as mybir
    from concourse.tile import TileContext

    f32 = mybir.dt.float32
    f32r = mybir.dt.float32r
    bf16 = mybir.dt.bfloat16

    nc = bacc.Bacc()

    x_d = nc.declare_dram_parameter("x", [BPC, S, I], f32, isOutput=False)
    p_d = nc.declare_dram_parameter("p", [BPC, S, I], f32, isOutput=False)
    c_d = nc.declare_dram_parameter("consts", [128, 262], f32r, isOutput=False)
    o_d = nc.declare_dram_parameter("o", [2 * BPC, S], f32, isOutput=True)

    with TileContext(nc) as tc:
        with (
            tc.tile_pool(name="const", bufs=1) as const,
            tc.tile_pool(name="inp", bufs=2) as inp,
            tc.tile_pool(name="hsb", bufs=4) as hsb,
            tc.tile_pool(name="pp", bufs=1) as pp,
            tc.tile_pool(name="wsum", bufs=2) as wsum,
            tc.tile_pool(name="ps_t", bufs=3, space="PSUM") as ps_t,
            tc.tile_pool(name="ps_h", bufs=3, space="PSUM") as ps_h,
            tc.tile_pool(name="ps_p", bufs=2, space="PSUM") as ps_p,
        ):
            c_t = const.tile([128, 262], f32r)
            nc.sync.dma_start(out=c_t[:], in_=c_d[:])
            w1_t = c_t[:, 0:128]
            wc_t = c_t[:, 128 : 128 + 2 * BPC]
            id_t = c_t[:, 132:260]                    # [128, 128] f32r identity
            b1_t = c_t[:, 260:261].bitcast(f32)
            b2_t = c_t[:, 261:262].bitcast(f32)

            # p_pad[(2b+f), 8 + s] = p^T values; zero pads at both ends so the
            # halo gather below never needs edge cases.
            p_pad = pp.tile([2 * BPC, PAD], f32)
            nc.vector.memset(p_pad[:, 0:8], 0.0)
            nc.vector.memset(p_pad[:, 8 + S : PAD], 0.0)

            for g in range(NG):
                s0 = g * GCHUNK
                # Raw layout: partition q holds 16 consecutive seq rows
                # (4 KB contiguous per partition -> cheap DMA descriptors).
                # Element (q, b, 64u + f) = x[b, s0 + 16q + u, f].
                xt = inp.tile([128, BPC, GCHUNK // 2], f32, tag="xt")
                pt = inp.tile([128, BPC, GCHUNK // 2], f32, tag="pt")
                for b in range(BPC):
                    nc.sync.dma_start(
                        out=xt[:, b],
                        in_=x_d[b, s0 : s0 + GCHUNK, :].rearrange(
                            "(q v) f -> q (v f)", q=128
                        ),
                    )
                    nc.sync.dma_start(
                        out=pt[:, b],
                        in_=p_d[b, s0 : s0 + GCHUNK, :].rearrange(
                            "(q v) f -> q (v f)", q=128
                        ),
                    )
                # h0c free layout (u, b, f): the (b, f) pair of each u-slice is
                # contiguous, so the transpose stationary is a single free dim.
                # Adds are split per 4-u-slice group so downstream transposes
                # unblock early; batch 0 on DVE, batch 1 on GpSimd.
                h0c = inp.tile([128, JG, BPC, I], f32r, tag="h0")
                for kk in range(GCHUNK // CHUNK):
                    js = slice(4 * kk, 4 * kk + 4)
                    nc.vector.tensor_add(
                        out=h0c[:, js, 0, :],
                        in0=xt[:, 0].rearrange("q (v f) -> q v f", f=I)[:, js],
                        in1=pt[:, 0].rearrange("q (v f) -> q v f", f=I)[:, js],
                    )
                    nc.gpsimd.tensor_add(
                        out=h0c[:, js, 1, :],
                        in0=xt[:, 1].rearrange("q (v f) -> q v f", f=I)[:, js],
                        in1=pt[:, 1].rearrange("q (v f) -> q v f", f=I)[:, js],
                    )

                for kk in range(GCHUNK // CHUNK):
                    # u-slices 4kk..4kk+3; transpose input [128, (b, f)] whose
                    # column q maps to seq s0 + 16q + u.
                    h0T_ps = ps_t.tile([128, CHUNK], f32r)
                    for ul in range(CHUNK // 128):
                        u = 4 * kk + ul
                        nc.tensor.transpose(
                            out=h0T_ps[:, 128 * ul : 128 * ul + 128],
                            in_=h0c[:, u].rearrange("p b f -> p (b f)"),
                            identity=id_t[:],
                        )
                    h0T = hsb.tile([128, CHUNK], f32r, tag="h0T")
                    nc.vector.tensor_copy(out=h0T[:], in_=h0T_ps[:])

                    hT_ps = ps_h.tile([128, CHUNK], f32)
                    nc.tensor.matmul(
                        out=hT_ps[:], lhsT=w1_t, rhs=h0T[:], start=True, stop=True
                    )
                    hT = hsb.tile([128, CHUNK], f32r, tag="hT")
                    nc.scalar.activation(
                        out=hT[:],
                        in_=hT_ps[:],
                        func=mybir.ActivationFunctionType.Relu,
                        bias=b1_t,
                    )

                    pT_ps = ps_p.tile([2 * BPC, CHUNK], f32)
                    nc.tensor.matmul(
                        out=pT_ps[:], lhsT=wc_t, rhs=hT[:], start=True, stop=True
                    )
                    # Un-permute while scattering: pT col (ul, q) is seq
                    # s0 + 16q + 4kk + ul -> p_pad col base + 16q + ul.
                    base = 8 + s0 + 4 * kk
                    nc.scalar.copy(
                        out=p_pad[:, base : base + 2048].rearrange(
                            "p (pp u) -> p u pp", u=16
                        )[:, 0:4, :],
                        in_=pT_ps.rearrange("p (u q) -> p u q", q=128),
                    )

            # Re-partition into halo layout Q[(bf,c), u] = p^T[bf, 128c + u - 8]
            # (zero-padded), bf in [0,4), c in [0,32), u in [0,144).
            q_t = wsum.tile([128, 144], f32, tag="q")
            nc.sync.dma_start(
                out=q_t[:, 0:128],
                in_=p_pad[:, 0 : 128 * 32].rearrange("p (c u) -> p c u", u=128),
            )
            nc.sync.dma_start(
                out=q_t[:, 128:136],
                in_=p_pad[:, 128 : 128 + 128 * 32].rearrange(
                    "p (c u) -> p c u", u=128
                )[:, :, 0:8],
            )
            nc.sync.dma_start(
                out=q_t[:, 136:144],
                in_=p_pad[:, 136 : 136 + 128 * 32].rearrange(
                    "p (c u) -> p c u", u=128
                )[:, :, 0:8],
            )

            # 17-wide window sum via doubling: ws[u] = sum_{d=0..16} Q[u+d].
            t2 = wsum.tile([128, 143], f32, tag="t2")
            nc.vector.tensor_add(out=t2[:], in0=q_t[:, 0:143], in1=q_t[:, 1:144])
            t4 = wsum.tile([128, 141], f32, tag="t4")
            nc.vector.tensor_add(out=t4[:], in0=t2[:, 0:141], in1=t2[:, 2:143])
            t8 = wsum.tile([128, 137], f32, tag="t8")
            nc.vector.tensor_add(out=t8[:], in0=t4[:, 0:137], in1=t4[:, 4:141])
            t16 = wsum.tile([128, 129], f32, tag="t16")
            nc.vector.tensor_add(out=t16[:], in0=t8[:, 0:129], in1=t8[:, 8:137])
            ws_t = wsum.tile([128, 128], f32, tag="ws")
            nc.vector.tensor_add(out=ws_t[:], in0=t16[:, 0:128], in1=q_t[:, 16:144])

            # + b2 (per-partition bias), then store: o[bf, 128c + u]
            ows = wsum.tile([128, 128], f32, tag="ows")
            nc.scalar.activation(
                out=ows[:],
                in_=ws_t[:],
                func=mybir.ActivationFunctionType.Identity,
                bias=b2_t,
            )
            nc.sync.dma_start(
                out=o_d[:, :].rearrange("p (c u) -> (p c) u", u=128), in_=ows[:]
            )

    nc.finalize()
    return nc


def _get_program():
    global _PROGRAM
    if _PROGRAM is None:
        _PROGRAM = _build_program()
    return _PROGRAM


def _host_inputs(W1, b1, Wv, W2, b2):
    """Pack the small replicated parameters into one [128, 198] f32 tensor."""
    W1 = np.asarray(W1, np.float32).reshape(I, H)
    Wc = (
        np.asarray(Wv, np.float32).reshape(H, H)
        @ np.asarray(W2, np.float32).reshape(H, O)
    ).astype(np.float32)
    consts = np.zeros((128, 262), np.float32)
    consts[:64, 0:64] = W1
    consts[64:, 64:128] = W1
    consts[:64, 128 : 128 + O] = Wc
    consts[64:, 128 + O : 128 + 2 * O] = Wc
    consts[:, 132:260] = np.eye(128, dtype=np.float32)
    consts[:, 260] = np.tile(np.asarray(b1, np.float32).reshape(H), BPC)
    pidx = np.arange(128)
    consts[:, 261] = np.asarray(b2, np.float32).reshape(O)[(pidx % 64) // 32]
    return consts


def _in_maps(x, pos_enc, W1, b1, Wv, W2, b2):
    x = np.asarray(x, np.float32)
    pos_enc = np.asarray(pos_enc, np.float32)
    consts = _host_inputs(W1, b1, Wv, W2, b2)
    in_maps = []
    for r in range(NCORES):
        sl = slice(r * BPC, (r + 1) * BPC)
        in_maps.append(
            {
                "x": np.ascontiguousarray(x[sl]),
                "p": np.ascontiguousarray(pos_enc[sl]),
                "consts": consts,
            }
        )
    return in_maps


def _assemble(results):
    out = np.empty((B, S, O), np.float32)
    for r in range(NCORES):
        o = np.asarray(results[r]["o"]).reshape(BPC, O, S)
        out[r * BPC : (r + 1) * BPC] = o.transpose(0, 2, 1)
    weights = np.ones((B, S, 1, 2 * A + 1), np.float32)
    return out, weights


def kernel(x, pos_enc, W1, b1, Wq, Wk, Wv, W2, b2):
    from concourse.bass_utils import run_bass_kernel_spmd

    in_maps = _in_maps(x, pos_enc, W1, b1, Wv, W2, b2)
    nc = _get_program()
    res = run_bass_kernel_spmd(nc, in_maps, list(range(NCORES))).results
    return _assemble(res)


# revision 19
# speedup vs baseline: 1.5188x; 1.0238x over previous
"""Trainium2 Bass kernel for a sliding-window self-attention block.

The reference network applies softmax over a singleton axis, so the attention
weights are identically 1.0 and the whole module reduces to:

    h   = relu((x + pos_enc) @ W1 + b1)            # [B, S, 64]
    p   = h @ (Wv @ W2)                            # [B, S, 2]
    out = sliding_window_sum(p, +-8, zero-pad) + b2
    weights = ones([B, S, 1, 17])

Sharding: data-parallel over batch, 2 batches per core on 8 NeuronCores.

Per-core dataflow (batches b0/b1 processed jointly on 128 partitions):
  - 4 big DMAs load x and pos_enc (1 MB each, both batches folded).
  - DVE adds -> h0 in [seq_on_partitions, (j, b, feat)] layout (float32r).
  - PE transposes [128 seq, (b,f)=128] tiles -> PSUM [(b,f), seq] (bf16
    identity as the moving operand: 1 cycle/row).
  - One block-diagonal f32r matmul per 512-chunk applies W1 to both batches,
    ACT relu(+b1) -> one more block-diag matmul applies Wc = Wv@W2 -> p^T.
  - p^T chunks gather into a zero-padded [4, 4240] buffer; 3 SBUF DMAs
    re-partition it into a [128(b,f,c), 144] halo layout; 5 log-tree DVE
    adds compute the 17-wide window sum; ACT adds b2; one DMA stores
    [4, 4096] = out^T per batch (host transposes the tiny result).
"""

import numpy as np

B, S, I, H, O = 16, 4096, 64, 64, 2
A = 8                 # atten_size; window = 2*A+1 = 17
NCORES = 8
BPC = B // NCORES     # batches per core = 2
CHUNK = 512           # seq positions per PSUM-stage chunk
NCHUNK = S // CHUNK   # 8
GCHUNK = 2048         # seq positions per DMA load chunk
NG = S // GCHUNK      # 2
JG = GCHUNK // 128    # 16 transpose tiles per load chunk
PAD = 4240            # 8 (left zero pad) + 4096 + 136 (right pad/tail slack)

_PROGRAM = None


def _build_program():
    import concourse.bacc as bacc
    import concourse.mybir as mybir
    from concourse.tile import TileContext

    f32 = mybir.dt.float32
    f32r = mybir.dt.float32r
    bf16 = mybir.dt.bfloat16

    nc = bacc.Bacc()

    x_d = nc.declare_dram_parameter("x", [BPC, S, I], f32, isOutput=False)
    p_d = nc.declare_dram_parameter("p", [BPC, S, I], f32, isOutput=False)
    c_d = nc.declare_dram_parameter("consts", [128, 262], f32r, isOutput=False)
    o_d = nc.declare_dram_parameter("o", [2 * BPC, S], f32, isOutput=True)

    with TileContext(nc) as tc:
        with (
            tc.tile_pool(name="const", bufs=1) as const,
            tc.tile_pool(name="inp", bufs=2) as inp,
            tc.tile_pool(name="hsb", bufs=4) as hsb,
            tc.tile_pool(name="pp", bufs=1) as pp,
            tc.tile_pool(name="wsum", bufs=2) as wsum,
            tc.tile_pool(name="ps_t", bufs=3, space="PSUM") as ps_t,
            tc.tile_pool(name="ps_h", bufs=3, space="PSUM") as ps_h,
            tc.tile_pool(name="ps_p", bufs=2, space="PSUM") as ps_p,
        ):
            c_t = const.tile([128, 262], f32r)
            nc.sync.dma_start(out=c_t[:], in_=c_d[:])
            w1_t = c_t[:, 0:128]
            wc_t = c_t[:, 128 : 128 + 2 * BPC]
            id_t = c_t[:, 132:260]                    # [128, 128] f32r identity
            b1_t = c_t[:, 260:261].bitcast(f32)
            b2_t = c_t[:, 261:262].bitcast(f32)

            # p_pad[(2b+f), 8 + s] = p^T values; zero pads at both ends so the
            # halo gather below never needs edge cases.
            p_pad = pp.tile([2 * BPC, PAD], f32)
            nc.vector.memset(p_pad[:, 0:8], 0.0)
            nc.vector.memset(p_pad[:, 8 + S : PAD], 0.0)

            for g in range(NG):
                s0 = g * GCHUNK
                # Raw layout: partition q holds 16 consecutive seq rows
                # (4 KB contiguous per partition -> cheap DMA descriptors).
                # Element (q, b, 64u + f) = x[b, s0 + 16q + u, f].
                xt = inp.tile([128, BPC, GCHUNK // 2], f32, tag="xt")
                pt = inp.tile([128, BPC, GCHUNK // 2], f32, tag="pt")
                # Half-granularity loads (256 KB each) so the first adds and
                # transposes unblock as early as possible.
                HC = GCHUNK // 4  # 1024 columns = 512 seq rows worth per half
                for h in range(2):
                    for b in range(BPC):
                        nc.sync.dma_start(
                            out=xt[:, b, h * HC : (h + 1) * HC],
                            in_=x_d[b, s0 : s0 + GCHUNK, :].rearrange(
                                "(q v) f -> q (v f)", q=128
                            )[:, h * HC : (h + 1) * HC],
                        )
                        nc.sync.dma_start(
                            out=pt[:, b, h * HC : (h + 1) * HC],
                            in_=p_d[b, s0 : s0 + GCHUNK, :].rearrange(
                                "(q v) f -> q (v f)", q=128
                            )[:, h * HC : (h + 1) * HC],
                        )
                # h0c free layout (u, b, f): the (b, f) pair of each u-slice is
                # contiguous, so the transpose stationary is a single free dim.
                # Adds are split per 4-u-slice group so downstream transposes
                # unblock early; batch 0 on DVE, batch 1 on GpSimd.
                h0c = inp.tile([128, JG, BPC, I], f32r, tag="h0")
                for kk in range(GCHUNK // CHUNK):
                    js = slice(4 * kk, 4 * kk + 4)
                    nc.vector.tensor_add(
                        out=h0c[:, js, 0, :],
                        in0=xt[:, 0].rearrange("q (v f) -> q v f", f=I)[:, js],
                        in1=pt[:, 0].rearrange("q (v f) -> q v f", f=I)[:, js],
                    )
                    nc.gpsimd.tensor_add(
                        out=h0c[:, js, 1, :],
                        in0=xt[:, 1].rearrange("q (v f) -> q v f", f=I)[:, js],
                        in1=pt[:, 1].rearrange("q (v f) -> q v f", f=I)[:, js],
                    )

                for kk in range(GCHUNK // CHUNK):
                    # u-slices 4kk..4kk+3; transpose input [128, (b, f)] whose
                    # column q maps to seq s0 + 16q + u.
                    h0T_ps = ps_t.tile([128, CHUNK], f32r)
                    for ul in range(CHUNK // 128):
                        u = 4 * kk + ul
                        nc.tensor.transpose(
                            out=h0T_ps[:, 128 * ul : 128 * ul + 128],
                            in_=h0c[:, u].rearrange("p b f -> p (b f)"),
                            identity=id_t[:],
                        )
                    h0T = hsb.tile([128, CHUNK], f32r, tag="h0T")
                    nc.vector.tensor_copy(out=h0T[:], in_=h0T_ps[:])

                    hT_ps = ps_h.tile([128, CHUNK], f32)
                    nc.tensor.matmul(
                        out=hT_ps[:], lhsT=w1_t, rhs=h0T[:], start=True, stop=True
                    )
                    hT = hsb.tile([128, CHUNK], f32r, tag="hT")
                    nc.scalar.activation(
                        out=hT[:],
                        in_=hT_ps[:],
                        func=mybir.ActivationFunctionType.Relu,
                        bias=b1_t,
                    )

                    pT_ps = ps_p.tile([2 * BPC, CHUNK], f32)
                    nc.tensor.matmul(
                        out=pT_ps[:], lhsT=wc_t, rhs=hT[:], start=True, stop=True
                    )
                    # Un-permute while scattering: pT col (ul, q) is seq
                    # s0 + 16q + 4kk + ul -> p_pad col base + 16q + ul.
                    base = 8 + s0 + 4 * kk
                    nc.scalar.copy(
                        out=p_pad[:, base : base + 2048].rearrange(
                            "p (pp u) -> p u pp", u=16
                        )[:, 0:4, :],
                        in_=pT_ps.rearrange("p (u q) -> p u q", q=128),
                    )

            # Re-partition into halo layout Q[(bf,c), u] = p^T[bf, 128c + u - 8]
            # (zero-padded), bf in [0,4), c in [0,32), u in [0,144).
            q_t = wsum.tile([128, 144], f32, tag="q")
            nc.sync.dma_start(
                out=q_t[:, 0:128],
                in_=p_pad[:, 0 : 128 * 32].rearrange("p (c u) -> p c u", u=128),
            )
            nc.sync.dma_start(
                out=q_t[:, 128:136],
                in_=p_pad[:, 128 : 128 + 128 * 32].rearrange(
                    "p (c u) -> p c u", u=128
                )[:, :, 0:8],
            )
            nc.sync.dma_start(
                out=q_t[:, 136:144],
                in_=p_pad[:, 136 : 136 + 128 * 32].rearrange(
                    "p (c u) -> p c u", u=128
                )[:, :, 0:8],
            )

            # 17-wide window sum via doubling: ws[u] = sum_{d=0..16} Q[u+d].
            t2 = wsum.tile([128, 143], f32, tag="t2")
            nc.vector.tensor_add(out=t2[:], in0=q_t[:, 0:143], in1=q_t[:, 1:144])
            t4 = wsum.tile([128, 141], f32, tag="t4")
            nc.vector.tensor_add(out=t4[:], in0=t2[:, 0:141], in1=t2[:, 2:143])
            t8 = wsum.tile([128, 137], f32, tag="t8")
            nc.vector.tensor_add(out=t8[:], in0=t4[:, 0:137], in1=t4[:, 4:141])
            t16 = wsum.tile([128, 129], f32, tag="t16")
            nc.vector.tensor_add(out=t16[:], in0=t8[:, 0:129], in1=t8[:, 8:137])
            ws_t = wsum.tile([128, 128], f32, tag="ws")
            nc.vector.tensor_add(out=ws_t[:], in0=t16[:, 0:128], in1=q_t[:, 16:144])

            # + b2 (per-partition bias), then store: o[bf, 128c + u]
            ows = wsum.tile([128, 128], f32, tag="ows")
            nc.scalar.activation(
                out=ows[:],
                in_=ws_t[:],
                func=mybir.ActivationFunctionType.Identity,
                bias=b2_t,
            )
            nc.sync.dma_start(
                out=o_d[:, :].rearrange("p (c u) -> (p c) u", u=128), in_=ows[:]
            )

    nc.finalize()
    return nc


def _get_program():
    global _PROGRAM
    if _PROGRAM is None:
        _PROGRAM = _build_program()
    return _PROGRAM


def _host_inputs(W1, b1, Wv, W2, b2):
    """Pack the small replicated parameters into one [128, 198] f32 tensor."""
    W1 = np.asarray(W1, np.float32).reshape(I, H)
    Wc = (
        np.asarray(Wv, np.float32).reshape(H, H)
        @ np.asarray(W2, np.float32).reshape(H, O)
    ).astype(np.float32)
    consts = np.zeros((128, 262), np.float32)
    consts[:64, 0:64] = W1
    consts[64:, 64:128] = W1
    consts[:64, 128 : 128 + O] = Wc
    consts[64:, 128 + O : 128 + 2 * O] = Wc
    consts[:, 132:260] = np.eye(128, dtype=np.float32)
    consts[:, 260] = np.tile(np.asarray(b1, np.float32).reshape(H), BPC)
    pidx = np.arange(128)
    consts[:, 261] = np.asarray(b2, np.float32).reshape(O)[(pidx % 64) // 32]
    return consts


def _in_maps(x, pos_enc, W1, b1, Wv, W2, b2):
    x = np.asarray(x, np.float32)
    pos_enc = np.asarray(pos_enc, np.float32)
    consts = _host_inputs(W1, b1, Wv, W2, b2)
    in_maps = []
    for r in range(NCORES):
        sl = slice(r * BPC, (r + 1) * BPC)
        in_maps.append(
            {
                "x": np.ascontiguousarray(x[sl]),
                "p": np.ascontiguousarray(pos_enc[sl]),
                "consts": consts,
            }
        )
    return in_maps


def _assemble(results):
    out = np.empty((B, S, O), np.float32)
    for r in range(NCORES):
        o = np.asarray(results[r]["o"]).reshape(BPC, O, S)
        out[r * BPC : (r + 1) * BPC] = o.transpose(0, 2, 1)
    weights = np.ones((B, S, 1, 2 * A + 1), np.float32)
    return out, weights


def kernel(x, pos_enc, W1, b1, Wq, Wk, Wv, W2, b2):
    from concourse.bass_utils import run_bass_kernel_spmd

    in_maps = _in_maps(x, pos_enc, W1, b1, Wv, W2, b2)
    nc = _get_program()
    res = run_bass_kernel_spmd(nc, in_maps, list(range(NCORES))).results
    return _assemble(res)
